# revision 64
# baseline (speedup 1.0000x reference)
"""Single-head attention kernel for TRN2, 8 NeuronCores.

Problem: hidden [4,4096,1024] fp32; Wq/Wk/Wv [1024,64]; out [4,4096,64]
  q,k,v = hidden @ W + b ; out = softmax(q k^T / 8) @ v

Sharding: 2 cores per batch; each core handles 2048 query rows but computes
K/V for the full 4096-row sequence of its batch (sequence parallelism over
the Q rows, K/V recomputed per core — no collectives). Host-side prep per
core: hidden[b] is rotated so this core's query rows are rows 0:2048, cast
to bf16 and laid out [128, S, NT] so partition p holds hid[s, 128t+p].
Softmax over keys is permutation-invariant, so the rotation is sound.

Design (cost model: matmul = moving-cols x 1cyc/row for bf16 any width and
f32r >=256 wide; PE is HW-decoded so many small matmuls are cheap):
  proj:   psum[KV|Q, 512 seq] = sum_t W_tile[:,t,:]^T @ hid[:, s0:s0+512, t]
          (hid/weights bf16; the [p][s][t] host layout gives 8 contiguous
          128-descriptor DMAs instead of 8192 descriptors)
  kT/qT:  [64, seq] f32r via DVE bias-add (f32r keeps score precision; the
          512-wide moving side still runs 1 cyc/row)
  vones:  [128, NK, 65] bf16; cols 0:64 = v (via PE transpose), col 64 = 1
  scores: sc psum [128 keys, 1024 q] per (tile, pair): 2 matmuls ap=512
  exp:    one ACT instr per sc tile -> wt bf16 [128, 1024], scale=1/8
          (ACT is the bottleneck engine: 64 x 1024 cols at 1.2 GHz)
  AV:     q-partition orientation: acc[128 q, 65] += wt[:,128j:]^T @ vones
          (ap=65 bf16 - half the PE cycles of the [65, q] orientation;
          col 64 accumulates softmax denominators; acc banks are pre-zeroed
          on DVE and accumulated with start=False because a start=True in a
          bank wipes other open accumulations in that bank)
  out:    acc * reciprocal(denom) -> natural-layout DMA, no transposes
  startup: pass-0 tiles 0-3 run split A/B exps fed by half-chunk DMAs and
          half-width projection groups so ACT starts ~9.5us in; junk warmup
          matmuls ramp the PE p-state during the first DMA.
  pieces: deferred projection work is interleaved 2 pieces/tile into pass 0
          (8 pieces per kv chunk; chunk c's kT piece pops at tile (8c-12)//2+4
          <= 4c-1, exactly before its first reader).  The emission ORDER is
          the dependency order - a reader emitted before its writer gets no
          semaphore and races on HW; CoreSim catches that deterministically.
  defer:  pass-0's avs for tiles 0..15 and 28..31 (plus its out_blocks) run
          inside pass 1's PE slack; pass-1's accumulators live in the pjp
          banks (idle by then) so pass-0's stay valid meanwhile.
"""

import numpy as np

E, S, H = 1024, 4096, 64
NT = E // 128  # 8 e-tiles
SQ = S // 2  # 2048 query rows per core
NK = S // 128  # 32 s_k tiles
N_CORES = 8
HEAD_T = 4  # startup tiles with split A/B exps
LAG = 4  # av trails exp by this many tiles
DEFER = 16  # pass-0 av groups from this tile on run inside pass 1's slack

_NC = None
LAST_RESULT = None  # BassKernelResults of the most recent run (for test.py)


def _build(dbg=False):
    from contextlib import ExitStack
    import concourse.tile as tile
    from concourse import bacc, mybir
    from concourse.masks import make_identity

    F32 = mybir.dt.float32
    F32R = mybir.dt.float32r
    BF16 = mybir.dt.bfloat16
    Exp = mybir.ActivationFunctionType.Exp
    Copy = mybir.ActivationFunctionType.Copy

    nc = bacc.Bacc("TRN2", target_bir_lowering=False, debug=False)
    if dbg:
        QTD = nc.dram_tensor("qtd", [64, SQ], F32, kind="ExternalOutput")
        KTD = nc.dram_tensor("ktd", [64, S], F32, kind="ExternalOutput")
        VOD = nc.dram_tensor("vod", [128, NK * (H + 1)], F32, kind="ExternalOutput")
        ACCD = nc.dram_tensor("accd", [128, 4 * 128], F32, kind="ExternalOutput")
    HIDT = nc.dram_tensor("hidt", [128, S, NT], BF16, kind="ExternalInput")
    WQ = nc.dram_tensor("wq", [128, NT, H], BF16, kind="ExternalInput")
    WKV = nc.dram_tensor("wkv", [128, NT, 2 * H], BF16, kind="ExternalInput")
    BQ = nc.dram_tensor("bq", [H, 1], F32, kind="ExternalInput")
    BKV = nc.dram_tensor("bkv", [2 * H, 1], F32, kind="ExternalInput")
    OUT = nc.dram_tensor("out", [SQ, H], F32, kind="ExternalOutput")

    with tile.TileContext(nc) as tc, ExitStack() as ctx:
        consts = ctx.enter_context(tc.tile_pool(name="consts", bufs=1))
        hidp = ctx.enter_context(tc.tile_pool(name="hid", bufs=1))
        stage = ctx.enter_context(tc.tile_pool(name="stage", bufs=3))
        wtp = ctx.enter_context(tc.tile_pool(name="wt", bufs=22))
        dbgp = ctx.enter_context(tc.tile_pool(name="dbg", bufs=1)) if dbg else None
        scp = ctx.enter_context(tc.tile_pool(name="scp", bufs=2, space="PSUM"))
        pjp = ctx.enter_context(tc.tile_pool(name="pjp", bufs=2, space="PSUM"))
        accp = ctx.enter_context(tc.tile_pool(name="accp", bufs=2, space="PSUM"))

        # ---- constants / SBUF layout ----
        wq_sb = consts.tile([128, NT, H], BF16)
        bq_sb = consts.tile([64, 1], F32)
        wkv_sb = consts.tile([128, NT, 2 * H], BF16)
        bkv_sb = consts.tile([128, 1], F32)
        identf = consts.tile([128, 128], F32)
        make_identity(nc, identf[:])
        identr = consts.tile([128, 128], F32R)
        nc.vector.tensor_copy(identr[:], identf[:])
        vones = consts.tile([128, NK, H + 1], BF16)
        ones32 = consts.tile([128, NK, 1], F32)
        nc.vector.memset(ones32[:], 1.0)
        nc.vector.tensor_copy(vones[:, :, 64:65], ones32[:])
        kT = consts.tile([64, S], F32R)
        qT = consts.tile([64, SQ], F32R)
        hidT_sb = hidp.tile([128, S, NT], BF16)

        # warm the Exp table early so the first real exp doesn't pay ~2.7us
        warm = consts.tile([1, 1], F32)
        nc.vector.memset(warm[:], 0.0)
        nc.scalar.activation(warm[:], warm[:], Exp)

        # warmup source for p-state ramp matmuls (zeros; results are junk)
        wsrc = consts.tile([128, 512], BF16)
        nc.vector.memset(wsrc[:], 0.0)

        # ---- DMAs on the sync/HWDGE queue (DMA engines are serial: order =
        # the startup chain).  Chunks 0/1 split in half so projections start
        # ~1.5us sooner; weights slot between them.
        def dma_hid(c0, cols):
            nc.sync.dma_start(
                hidT_sb[:, c0 : c0 + cols, :], HIDT[:, c0 : c0 + cols, :]
            )

        nc.sync.dma_start(wq_sb[:], WQ[:])
        nc.sync.dma_start(wkv_sb[:], WKV[:])
        dma_hid(0, 256)
        dma_hid(256, 256)
        nc.sync.dma_start(bq_sb[:], BQ[:])
        nc.sync.dma_start(bkv_sb[:], BKV[:])
        dma_hid(512, 256)
        dma_hid(768, 256)
        for c in range(2, 8):
            dma_hid(512 * c, 512)

        # ---- PE warmup: ramp the p-state while DMA c0 lands ----
        for _ in range(8):
            junk = pjp.tile([128, 512], F32, tag="pj", name="warm")
            nc.tensor.matmul(junk[:], wsrc[:, 0:128], wsrc[:], start=True, stop=True)

        # ---- projection helpers ----
        def q_mm(box, g, t0, t1, h0=0, h1=512):
            if "pq" not in box:
                box["pq"] = pjp.tile([64, 512], F32, tag="pj", name="pq")
            for t in range(t0, t1):
                nc.tensor.matmul(
                    box["pq"][:, h0:h1],
                    wq_sb[:, t, :],
                    hidT_sb[:, 512 * g + h0 : 512 * g + h1, t],
                    start=(t == 0),
                    stop=(t == NT - 1),
                )

        def q_add(box, g):
            nc.vector.tensor_scalar_add(
                qT[:, 512 * g : 512 * (g + 1)], box["pq"][:], bq_sb[:]
            )

        def q_group_pieces(g):
            box = {}
            return [
                lambda: q_mm(box, g, 0, 3),
                lambda: q_mm(box, g, 3, 6),
                lambda: q_mm(box, g, 6, 8),
                lambda: q_add(box, g),
            ]

        def kv_mm(box, c, t0, t1, h0=0, h1=512):
            if "pkv" not in box:
                box["pkv"] = pjp.tile([128, 512], F32, tag="pj", name="pkv")
            for t in range(t0, t1):
                nc.tensor.matmul(
                    box["pkv"][:, h0:h1],
                    wkv_sb[:, t, :],
                    hidT_sb[:, 512 * c + h0 : 512 * c + h1, t],
                    start=(t == 0),
                    stop=(t == NT - 1),
                )

        def k_add(box, c, h0=0, h1=512):
            nc.vector.tensor_scalar_add(
                kT[:, 512 * c + h0 : 512 * c + h1],
                box["pkv"][0:64, h0:h1],
                bkv_sb[0:64, :],
            )

        def v_pieces(box, c):
            def p_v():
                vstg = stage.tile([64, 512], F32R, tag="vstg")
                nc.vector.tensor_scalar_add(
                    vstg[:], box["pkv"][64:128, :], bkv_sb[64:128, :]
                )
                box["vstg"] = vstg

            def p_tp():
                pv = pjp.tile([128, 4, 64], F32R, tag="pj", name="pv")
                for j in range(4):
                    nc.tensor.transpose(
                        pv[:, j, :],
                        box["vstg"][:, 128 * j : 128 * (j + 1)],
                        identr[0:64, 0:64],
                    )
                box["pv"] = pv

            def p_tpc():
                nc.vector.tensor_copy(vones[:, 4 * c : 4 * c + 4, 0:64], box["pv"][:])

            return [p_v, p_tp, p_tpc]

        def kv_chunk_pieces(c):
            box = {}
            return [
                lambda: kv_mm(box, c, 0, 2),
                lambda: kv_mm(box, c, 2, 4),
                lambda: kv_mm(box, c, 4, 6),
                lambda: kv_mm(box, c, 6, 8),
                lambda: k_add(box, c),
            ] + v_pieces(box, c)

        # ---- startup: kv chunk 0 + q group 0, half-width so the first
        # scores fire as soon as the first half-chunk DMA lands ----
        kv0 = {}
        qg0 = {}
        q_mm(qg0, 0, 0, 8, 0, 256)
        kv_mm(kv0, 0, 0, 8, 0, 256)
        k_add(kv0, 0, 0, 256)
        q_mm(qg0, 0, 0, 8, 256, 512)
        q_add(qg0, 0)
        kv_mm(kv0, 0, 0, 8, 256, 512)
        k_add(kv0, 0, 256, 512)
        for fn in v_pieces(kv0, 0):
            fn()

        # deferred pieces: kv chunks 1-7 then q groups 2/3 (pass-1 only),
        # popped 2/tile — chunk c's kT piece is emitted just before its
        # first reader sc(4c) and after its DMA has landed.  Chunk 7's
        # v pieces go last: its avs are deferred into pass 1, so moving
        # qg2/qg3 ahead of them un-stalls the pass transition.
        pieces = []
        for c in (1, 2, 3, 4, 5, 6):
            pieces += kv_chunk_pieces(c)
        c7 = kv_chunk_pieces(7)
        pieces += c7[:5]
        pieces += q_group_pieces(2)
        pieces += q_group_pieces(3)
        pieces += c7[5:]

        def attn_pass(
            P, accA, accB, interleave, head=False, tail=None, defer=False, extra=None
        ):
            """One sweep over all 32 key tiles for q columns [1024P, 1024P+1024)."""
            wts = {}
            scs = {}

            def sc_mm(t, h, sc):
                nc.tensor.matmul(
                    sc[:],
                    kT[:, 128 * t : 128 * (t + 1)],
                    qT[:, 1024 * P + 512 * h : 1024 * P + 512 * (h + 1)],
                    start=True,
                    stop=True,
                )

            def piece():
                if interleave and pieces:
                    pieces.pop(0)()

            # pre-zero both acc banks on DVE (matmul start=True cannot be
            # used per-region: it wipes other open accumulations in the same
            # bank; a PE zeroing matmul also races the previous pass's
            # out_block reads), then accumulate in place with start=False.
            for acc in (accA, accB):
                nc.vector.memset(acc[:], 0.0)

            def av_js(t, j0, j1):
                wt = wts[t]
                for j in range(j0, j1):
                    acc = accA if j < 4 else accB
                    nc.tensor.matmul(
                        acc[:, j % 4, 0:65],
                        wt[:, 128 * j : 128 * (j + 1)],
                        vones[:, t, :],
                        start=False,
                        stop=(t == NK - 1),
                        skip_group_check=True,
                    )

            def av(t):
                av_js(t, 0, 8)
                del wts[t]

            t0 = 0
            if head:
                # A halves of tiles 0..7 need only q group 0 + kT; tiles 4-7
                # are gated by chunk 1's kT piece (popped between A3 and A4),
                # which keeps ACT fed while q group 1 (for the B halves) and
                # the chunk-1 DMA complete.  B tiles carry 2 pops each.
                half = {}

                def scA(t):
                    half[t, 0] = scp.tile([128, 512], F32, tag="sc", name=f"scA{t}")
                    wts[t] = wtp.tile([128, 1024], BF16, tag="wt", name=f"wt{t}")
                    sc_mm(t, 0, half[t, 0])
                    nc.scalar.activation(
                        wts[t][:, 0:512], half[t, 0][:], Exp, scale=0.125
                    )

                def scB(t):
                    half[t, 1] = scp.tile([128, 512], F32, tag="sc", name=f"scB{t}")
                    sc_mm(t, 1, half[t, 1])
                    nc.scalar.activation(
                        wts[t][:, 512:1024], half[t, 1][:], Exp, scale=0.125
                    )

                for t in range(HEAD_T):
                    scA(t)
                # qg1 in half-chunk groups: the first half overlaps the
                # c1h2 DMA instead of waiting for the whole chunk
                qg1 = {}
                q_mm(qg1, 1, 0, 8, 0, 256)
                q_mm(qg1, 1, 0, 8, 256, 512)
                q_add(qg1, 1)
                for t in range(HEAD_T):
                    scB(t)
                    piece()
                    piece()
                t0 = HEAD_T

            for t in range(t0, NK):
                scs[t] = scp.tile([128, 1024], F32, tag="sc", name=f"sc{t}")
                nc.tensor.matmul(
                    scs[t][:, 0:512],
                    kT[:, 128 * t : 128 * (t + 1)],
                    qT[:, 1024 * P : 1024 * P + 512],
                    start=True,
                    stop=True,
                )
                nc.tensor.matmul(
                    scs[t][:, 512:1024],
                    kT[:, 128 * t : 128 * (t + 1)],
                    qT[:, 1024 * P + 512 : 1024 * P + 1024],
                    start=True,
                    stop=True,
                )
                wt = wtp.tile([128, 1024], BF16, tag="wt", name=f"wt{t}")
                nc.scalar.activation(wt[:], scs[t][:], Exp, scale=0.125)
                wts[t] = wt
                scs.pop(t - LAG, None)
                piece()
                if extra:
                    extra.pop(0)()
                if t >= LAG and (not defer or DEFER <= t - LAG < 24):
                    av(t - LAG)
                piece()
            if defer:
                # defer the avs of the piece-congested window (tiles 0..15)
                # AND the final-tile flush into the next pass, which has PE
                # slack and no pieces (the flush would stall PE on this
                # pass's last exps right when the next pass's scores could
                # run).  Keep tiles 28..31 last so each region's stop flag
                # still closes its accumulation group.
                return [
                    (lambda tt: lambda: av(tt))(t)
                    for t in list(range(DEFER)) + list(range(24, NK))
                ]
            if tail is None:
                for t in range(NK - LAG, NK):
                    av(t)
            else:
                # finish accB's accumulation first so its out_block (on ACT)
                # runs in parallel with accA's remaining avs + DVE out_block
                cb_b, cb_a = tail
                for t in range(NK - LAG, NK):
                    av_js(t, 4, 8)
                cb_b()
                for t in range(NK - LAG, NK):
                    av_js(t, 0, 4)
                cb_a()

        def out_block(acc, blk, on_act=False):
            # blk in 0..3: output rows 512*blk .. 512*blk+512.  The final
            # (tail) blocks split the multiplies between ACT and DVE.
            res = stage.tile([128, 4, H], F32, tag="res")
            for j in range(4):
                rec = stage.tile([128, 1], F32, tag="rec")
                nc.vector.reciprocal(rec[:], acc[:, j, 64:65])
                if on_act:
                    nc.scalar.activation(
                        res[:, j, :], acc[:, j, 0:64], Copy, scale=rec[:]
                    )
                else:
                    nc.vector.tensor_scalar_mul(res[:, j, :], acc[:, j, 0:64], rec[:])
            nc.sync.dma_start(
                OUT[512 * blk : 512 * (blk + 1), :].rearrange("(j p) c -> p j c", p=128),
                res[:],
            )

        accA = accp.tile([128, 4, 128], F32, tag="acc", name="acc0A")
        accB = accp.tile([128, 4, 128], F32, tag="acc", name="acc0B")
        av_defer = attn_pass(0, accA, accB, interleave=True, head=True, defer=True)
        assert not pieces, f"{len(pieces)} deferred pieces never emitted"
        if dbg:
            qtd = dbgp.tile([64, SQ], F32, tag="qtd")
            nc.vector.tensor_copy(qtd[:], qT[:])
            nc.sync.dma_start(QTD[:], qtd[:])
            ktd = dbgp.tile([64, S], F32, tag="ktd")
            nc.vector.tensor_copy(ktd[:], kT[:])
            nc.sync.dma_start(KTD[:], ktd[:])
            vod = dbgp.tile([128, NK * (H + 1)], F32, tag="vod")
            nc.vector.tensor_copy(vod[:], vones[:].rearrange("p a b -> p (a b)"))
            nc.sync.dma_start(VOD[:], vod[:])
            accd = dbgp.tile([128, 4 * 128], F32, tag="accd")
            nc.vector.tensor_copy(accd[:], accA[:].rearrange("p a b -> p (a b)"))
            nc.sync.dma_start(ACCD[:], accd[:])
        av_defer.append(lambda: out_block(accA, 0))
        av_defer.append(lambda: out_block(accB, 1))

        # pass-1 accumulators live in the (now idle) pjp banks so pass 0's
        # stay valid while its deferred avs drain inside pass 1
        acc1A = pjp.tile([128, 4, 128], F32, tag="pj", name="acc1A")
        acc1B = pjp.tile([128, 4, 128], F32, tag="pj", name="acc1B")
        res8 = stage.tile([128, 8, H], F32, tag="res8", name="res8")

        def tail_half(acc, r0, dve_only):
            # reciprocals first, then multiply pairs with the pair's output
            # DMA issued immediately (overlapping the DMA-issue pipeline with
            # the remaining normalize work).  The first (critical-path) half
            # keeps everything on DVE — one queue, no cross-engine semaphore
            # hops; the second half splits DVE/ACT for parallelism.
            rec4 = stage.tile([128, 4], F32, tag="rec4", name=f"rec4_{r0}")
            for j in range(4):
                nc.vector.reciprocal(rec4[:, j : j + 1], acc[:, j, 64:65])
            for jp in (0, 2):
                nc.vector.tensor_scalar_mul(
                    res8[:, r0 + jp, :], acc[:, jp, 0:64], rec4[:, jp : jp + 1]
                )
                if dve_only:
                    nc.vector.tensor_scalar_mul(
                        res8[:, r0 + jp + 1, :],
                        acc[:, jp + 1, 0:64],
                        rec4[:, jp + 1 : jp + 2],
                    )
                else:
                    nc.scalar.activation(
                        res8[:, r0 + jp + 1, :],
                        acc[:, jp + 1, 0:64],
                        Copy,
                        scale=rec4[:, jp + 1 : jp + 2],
                    )
                row = 1024 + 128 * (r0 + jp)
                nc.sync.dma_start(
                    OUT[row : row + 256, :].rearrange("(j p) c -> p j c", p=128),
                    res8[:, r0 + jp : r0 + jp + 2, :],
                )

        def tail_b():
            tail_half(acc1B, 4, dve_only=True)

        def tail_a():
            tail_half(acc1A, 0, dve_only=False)

        attn_pass(
            1, acc1A, acc1B, interleave=False, extra=av_defer, tail=(tail_b, tail_a)
        )
        assert not av_defer, f"{len(av_defer)} deferred avs never emitted"

    nc.compile()
    return nc


def kernel(hidden_states, Wq, bq, Wk, bk, Wv, bv):
    global _NC, LAST_RESULT
    import ml_dtypes
    from concourse.bass_utils import run_bass_kernel_spmd

    BF = ml_dtypes.bfloat16
    hidden_states = np.asarray(hidden_states, dtype=np.float32)
    Wq = np.asarray(Wq, dtype=np.float32)
    Wk = np.asarray(Wk, dtype=np.float32)
    Wv = np.asarray(Wv, dtype=np.float32)
    bq = np.asarray(bq, dtype=np.float32)
    bk = np.asarray(bk, dtype=np.float32)
    bv = np.asarray(bv, dtype=np.float32)
    B = hidden_states.shape[0]
    assert hidden_states.shape == (4, S, E), hidden_states.shape

    if _NC is None:
        _NC = _build()

    wkv = np.concatenate([Wk, Wv], axis=1)  # [E, 128]
    wkv_t = np.ascontiguousarray(
        wkv.reshape(NT, 128, 2 * H).transpose(1, 0, 2).astype(BF)
    )
    wq_t = np.ascontiguousarray(Wq.reshape(NT, 128, H).transpose(1, 0, 2).astype(BF))
    bkv = np.concatenate([bk, bv]).reshape(2 * H, 1).copy()
    bq1 = bq.reshape(H, 1).copy()

    in_maps = []
    for core in range(N_CORES):
        b, half = divmod(core, 2)
        q0 = half * SQ
        hid_rot = np.roll(hidden_states[b], -q0, axis=0)  # [S, E]
        hidt = np.ascontiguousarray(
            hid_rot.reshape(S, NT, 128).transpose(2, 0, 1).astype(BF)
        )  # [128, S, NT]
        in_maps.append({"hidt": hidt, "wkv": wkv_t, "wq": wq_t, "bkv": bkv, "bq": bq1})

    LAST_RESULT = run_bass_kernel_spmd(_NC, in_maps, core_ids=list(range(N_CORES)))
    out = np.empty((B, S, H), np.float32)
    for core in range(N_CORES):
        b, half = divmod(core, 2)
        q0 = half * SQ
        out[b, q0 : q0 + SQ] = LAST_RESULT.results[core]["out"]
    return out


# revision 65
# speedup vs baseline: 1.0049x; 1.0049x over previous
"""Single-head attention kernel for TRN2, 8 NeuronCores.

Problem: hidden [4,4096,1024] fp32; Wq/Wk/Wv [1024,64]; out [4,4096,64]
  q,k,v = hidden @ W + b ; out = softmax(q k^T / 8) @ v

Sharding: 2 cores per batch; each core handles 2048 query rows but computes
K/V for the full 4096-row sequence of its batch (sequence parallelism over
the Q rows, K/V recomputed per core — no collectives). Host-side prep per
core: hidden[b] is rotated so this core's query rows are rows 0:2048, cast
to bf16 and laid out [128, S, NT] so partition p holds hid[s, 128t+p].
Softmax over keys is permutation-invariant, so the rotation is sound.

Design (cost model: matmul = moving-cols x 1cyc/row for bf16 any width and
f32r >=256 wide; PE is HW-decoded so many small matmuls are cheap):
  proj:   psum[KV|Q, 512 seq] = sum_t W_tile[:,t,:]^T @ hid[:, s0:s0+512, t]
          (hid/weights bf16; the [p][s][t] host layout gives 8 contiguous
          128-descriptor DMAs instead of 8192 descriptors)
  kT/qT:  [64, seq] f32r via DVE bias-add (f32r keeps score precision; the
          512-wide moving side still runs 1 cyc/row)
  vones:  [128, NK, 65] bf16; cols 0:64 = v (via PE transpose), col 64 = 1
  scores: sc psum [128 keys, 1024 q] per (tile, pair): 2 matmuls ap=512
  exp:    one ACT instr per sc tile -> wt bf16 [128, 1024], scale=1/8
          (ACT is the bottleneck engine: 64 x 1024 cols at 1.2 GHz)
  AV:     q-partition orientation: acc[128 q, 65] += wt[:,128j:]^T @ vones
          (ap=65 bf16 - half the PE cycles of the [65, q] orientation;
          col 64 accumulates softmax denominators; acc banks are pre-zeroed
          on DVE and accumulated with start=False because a start=True in a
          bank wipes other open accumulations in that bank)
  out:    acc * reciprocal(denom) -> natural-layout DMA, no transposes
  startup: pass-0 tiles 0-3 run split A/B exps fed by half-chunk DMAs and
          half-width projection groups so ACT starts ~9.5us in; junk warmup
          matmuls ramp the PE p-state during the first DMA.
  pieces: deferred projection work is interleaved 2 pieces/tile into pass 0
          (8 pieces per kv chunk; chunk c's kT piece pops at tile (8c-12)//2+4
          <= 4c-1, exactly before its first reader).  The emission ORDER is
          the dependency order - a reader emitted before its writer gets no
          semaphore and races on HW; CoreSim catches that deterministically.
  defer:  pass-0's avs for tiles 0..15 and 28..31 (plus its out_blocks) run
          inside pass 1's PE slack; pass-1's accumulators live in the pjp
          banks (idle by then) so pass-0's stay valid meanwhile.
"""

import numpy as np

E, S, H = 1024, 4096, 64
NT = E // 128  # 8 e-tiles
SQ = S // 2  # 2048 query rows per core
NK = S // 128  # 32 s_k tiles
N_CORES = 8
HEAD_T = 4  # startup tiles with split A/B exps
LAG = 4  # av trails exp by this many tiles
DEFER = 16  # pass-0 av groups from this tile on run inside pass 1's slack

_NC = None
LAST_RESULT = None  # BassKernelResults of the most recent run (for test.py)


def _build(dbg=False):
    from contextlib import ExitStack
    import concourse.tile as tile
    from concourse import bacc, mybir
    from concourse.masks import make_identity

    F32 = mybir.dt.float32
    F32R = mybir.dt.float32r
    BF16 = mybir.dt.bfloat16
    Exp = mybir.ActivationFunctionType.Exp
    Copy = mybir.ActivationFunctionType.Copy

    nc = bacc.Bacc("TRN2", target_bir_lowering=False, debug=False)
    if dbg:
        QTD = nc.dram_tensor("qtd", [64, SQ], F32, kind="ExternalOutput")
        KTD = nc.dram_tensor("ktd", [64, S], F32, kind="ExternalOutput")
        VOD = nc.dram_tensor("vod", [128, NK * (H + 1)], F32, kind="ExternalOutput")
        ACCD = nc.dram_tensor("accd", [128, 4 * 128], F32, kind="ExternalOutput")
    HIDT = nc.dram_tensor("hidt", [128, S, NT], BF16, kind="ExternalInput")
    WQ = nc.dram_tensor("wq", [128, NT, H], BF16, kind="ExternalInput")
    WKV = nc.dram_tensor("wkv", [128, NT, 2 * H], BF16, kind="ExternalInput")
    BQ = nc.dram_tensor("bq", [H, 1], F32, kind="ExternalInput")
    BKV = nc.dram_tensor("bkv", [2 * H, 1], F32, kind="ExternalInput")
    OUT = nc.dram_tensor("out", [SQ, H], F32, kind="ExternalOutput")

    with tile.TileContext(nc) as tc, ExitStack() as ctx:
        consts = ctx.enter_context(tc.tile_pool(name="consts", bufs=1))
        hidp = ctx.enter_context(tc.tile_pool(name="hid", bufs=1))
        stage = ctx.enter_context(tc.tile_pool(name="stage", bufs=3))
        wtp = ctx.enter_context(tc.tile_pool(name="wt", bufs=22))
        dbgp = ctx.enter_context(tc.tile_pool(name="dbg", bufs=1)) if dbg else None
        scp = ctx.enter_context(tc.tile_pool(name="scp", bufs=2, space="PSUM"))
        pjp = ctx.enter_context(tc.tile_pool(name="pjp", bufs=2, space="PSUM"))
        accp = ctx.enter_context(tc.tile_pool(name="accp", bufs=2, space="PSUM"))

        # ---- constants / SBUF layout ----
        wq_sb = consts.tile([128, NT, H], BF16)
        bq_sb = consts.tile([64, 1], F32)
        wkv_sb = consts.tile([128, NT, 2 * H], BF16)
        bkv_sb = consts.tile([128, 1], F32)
        identf = consts.tile([128, 128], F32)
        make_identity(nc, identf[:])
        identr = consts.tile([128, 128], F32R)
        nc.vector.tensor_copy(identr[:], identf[:])
        vones = consts.tile([128, NK, H + 1], BF16)
        ones32 = consts.tile([128, NK, 1], F32)
        nc.vector.memset(ones32[:], 1.0)
        nc.vector.tensor_copy(vones[:, :, 64:65], ones32[:])
        kT = consts.tile([64, S], F32R)
        qT = consts.tile([64, SQ], F32R)
        hidT_sb = hidp.tile([128, S, NT], BF16)

        # warm the Exp table early so the first real exp doesn't pay ~2.7us
        warm = consts.tile([1, 1], F32)
        nc.vector.memset(warm[:], 0.0)
        nc.scalar.activation(warm[:], warm[:], Exp)

        # warmup source for p-state ramp matmuls (zeros; results are junk)
        wsrc = consts.tile([128, 512], BF16)
        nc.vector.memset(wsrc[:], 0.0)

        # ---- DMAs on the sync/HWDGE queue (DMA engines are serial: order =
        # the startup chain).  Chunks 0/1 split in half so projections start
        # ~1.5us sooner; weights slot between them.
        def dma_hid(c0, cols):
            nc.sync.dma_start(
                hidT_sb[:, c0 : c0 + cols, :], HIDT[:, c0 : c0 + cols, :]
            )

        nc.sync.dma_start(wq_sb[:], WQ[:])
        nc.sync.dma_start(wkv_sb[:], WKV[:])
        dma_hid(0, 256)
        dma_hid(256, 256)
        nc.sync.dma_start(bq_sb[:], BQ[:])
        nc.sync.dma_start(bkv_sb[:], BKV[:])
        dma_hid(512, 256)
        dma_hid(768, 256)
        for c in range(2, 8):
            dma_hid(512 * c, 512)

        # ---- PE warmup: ramp the p-state while DMA c0 lands ----
        for _ in range(8):
            junk = pjp.tile([128, 512], F32, tag="pj", name="warm")
            nc.tensor.matmul(junk[:], wsrc[:, 0:128], wsrc[:], start=True, stop=True)

        # ---- projection helpers ----
        def q_mm(box, g, t0, t1, h0=0, h1=512):
            if "pq" not in box:
                box["pq"] = pjp.tile([64, 512], F32, tag="pj", name="pq")
            for t in range(t0, t1):
                nc.tensor.matmul(
                    box["pq"][:, h0:h1],
                    wq_sb[:, t, :],
                    hidT_sb[:, 512 * g + h0 : 512 * g + h1, t],
                    start=(t == 0),
                    stop=(t == NT - 1),
                )

        def q_add(box, g):
            nc.vector.tensor_scalar_add(
                qT[:, 512 * g : 512 * (g + 1)], box["pq"][:], bq_sb[:]
            )

        def q_group_pieces(g):
            box = {}
            return [
                lambda: q_mm(box, g, 0, 3),
                lambda: q_mm(box, g, 3, 6),
                lambda: q_mm(box, g, 6, 8),
                lambda: q_add(box, g),
            ]

        def kv_mm(box, c, t0, t1, h0=0, h1=512):
            if "pkv" not in box:
                box["pkv"] = pjp.tile([128, 512], F32, tag="pj", name="pkv")
            for t in range(t0, t1):
                nc.tensor.matmul(
                    box["pkv"][:, h0:h1],
                    wkv_sb[:, t, :],
                    hidT_sb[:, 512 * c + h0 : 512 * c + h1, t],
                    start=(t == 0),
                    stop=(t == NT - 1),
                )

        def k_add(box, c, h0=0, h1=512):
            nc.vector.tensor_scalar_add(
                kT[:, 512 * c + h0 : 512 * c + h1],
                box["pkv"][0:64, h0:h1],
                bkv_sb[0:64, :],
            )

        def v_pieces(box, c):
            def p_v():
                vstg = stage.tile([64, 512], F32R, tag="vstg")
                nc.vector.tensor_scalar_add(
                    vstg[:], box["pkv"][64:128, :], bkv_sb[64:128, :]
                )
                box["vstg"] = vstg

            def p_tp():
                pv = pjp.tile([128, 4, 64], F32R, tag="pj", name="pv")
                for j in range(4):
                    nc.tensor.transpose(
                        pv[:, j, :],
                        box["vstg"][:, 128 * j : 128 * (j + 1)],
                        identr[0:64, 0:64],
                    )
                box["pv"] = pv

            def p_tpc():
                nc.vector.tensor_copy(vones[:, 4 * c : 4 * c + 4, 0:64], box["pv"][:])

            return [p_v, p_tp, p_tpc]

        def kv_chunk_pieces(c):
            box = {}
            return [
                lambda: kv_mm(box, c, 0, 2),
                lambda: kv_mm(box, c, 2, 4),
                lambda: kv_mm(box, c, 4, 6),
                lambda: kv_mm(box, c, 6, 8),
                lambda: k_add(box, c),
            ] + v_pieces(box, c)

        # ---- startup: kv chunk 0 + q group 0, half-width so the first
        # scores fire as soon as the first half-chunk DMA lands ----
        kv0 = {}
        qg0 = {}
        q_mm(qg0, 0, 0, 8, 0, 256)
        kv_mm(kv0, 0, 0, 8, 0, 256)
        k_add(kv0, 0, 0, 256)
        q_mm(qg0, 0, 0, 8, 256, 512)
        q_add(qg0, 0)
        kv_mm(kv0, 0, 0, 8, 256, 512)
        k_add(kv0, 0, 256, 512)
        for fn in v_pieces(kv0, 0):
            fn()

        # deferred pieces: kv chunks 1-7 then q groups 2/3 (pass-1 only),
        # popped 2/tile — chunk c's kT piece is emitted just before its
        # first reader sc(4c) and after its DMA has landed.  Chunk 7's
        # v pieces go last: its avs are deferred into pass 1, so moving
        # qg2/qg3 ahead of them un-stalls the pass transition.
        pieces = []
        for c in (1, 2, 3, 4, 5, 6):
            pieces += kv_chunk_pieces(c)
        c7 = kv_chunk_pieces(7)
        pieces += c7[:5]
        pieces += q_group_pieces(2)
        pieces += q_group_pieces(3)
        pieces += c7[5:]

        def attn_pass(
            P, accA, accB, interleave, head=False, tail=None, defer=False, extra=None
        ):
            """One sweep over all 32 key tiles for q columns [1024P, 1024P+1024)."""
            wts = {}
            scs = {}

            def sc_mm(t, h, sc):
                nc.tensor.matmul(
                    sc[:],
                    kT[:, 128 * t : 128 * (t + 1)],
                    qT[:, 1024 * P + 512 * h : 1024 * P + 512 * (h + 1)],
                    start=True,
                    stop=True,
                )

            def piece():
                if interleave and pieces:
                    pieces.pop(0)()

            # pre-zero both acc banks on DVE (matmul start=True cannot be
            # used per-region: it wipes other open accumulations in the same
            # bank; a PE zeroing matmul also races the previous pass's
            # out_block reads), then accumulate in place with start=False.
            for acc in (accA, accB):
                nc.vector.memset(acc[:], 0.0)

            def av_js(t, j0, j1):
                wt = wts[t]
                for j in range(j0, j1):
                    acc = accA if j < 4 else accB
                    nc.tensor.matmul(
                        acc[:, j % 4, 0:65],
                        wt[:, 128 * j : 128 * (j + 1)],
                        vones[:, t, :],
                        start=False,
                        stop=(t == NK - 1),
                        skip_group_check=True,
                    )

            def av(t):
                av_js(t, 0, 8)
                del wts[t]

            t0 = 0
            if head:
                # A halves of tiles 0..7 need only q group 0 + kT; tiles 4-7
                # are gated by chunk 1's kT piece (popped between A3 and A4),
                # which keeps ACT fed while q group 1 (for the B halves) and
                # the chunk-1 DMA complete.  B tiles carry 2 pops each.
                half = {}

                def scA(t):
                    half[t, 0] = scp.tile([128, 512], F32, tag="sc", name=f"scA{t}")
                    wts[t] = wtp.tile([128, 1024], BF16, tag="wt", name=f"wt{t}")
                    sc_mm(t, 0, half[t, 0])
                    nc.scalar.activation(
                        wts[t][:, 0:512], half[t, 0][:], Exp, scale=0.125
                    )

                def scB(t):
                    half[t, 1] = scp.tile([128, 512], F32, tag="sc", name=f"scB{t}")
                    sc_mm(t, 1, half[t, 1])
                    nc.scalar.activation(
                        wts[t][:, 512:1024], half[t, 1][:], Exp, scale=0.125
                    )

                for t in range(HEAD_T):
                    scA(t)
                # qg1 in half-chunk groups: the first half overlaps the
                # c1h2 DMA instead of waiting for the whole chunk
                qg1 = {}
                q_mm(qg1, 1, 0, 8, 0, 256)
                q_mm(qg1, 1, 0, 8, 256, 512)
                q_add(qg1, 1)
                for t in range(HEAD_T):
                    scB(t)
                    piece()
                    piece()
                t0 = HEAD_T

            for t in range(t0, NK):
                scs[t] = scp.tile([128, 1024], F32, tag="sc", name=f"sc{t}")
                nc.tensor.matmul(
                    scs[t][:, 0:512],
                    kT[:, 128 * t : 128 * (t + 1)],
                    qT[:, 1024 * P : 1024 * P + 512],
                    start=True,
                    stop=True,
                )
                nc.tensor.matmul(
                    scs[t][:, 512:1024],
                    kT[:, 128 * t : 128 * (t + 1)],
                    qT[:, 1024 * P + 512 : 1024 * P + 1024],
                    start=True,
                    stop=True,
                )
                wt = wtp.tile([128, 1024], BF16, tag="wt", name=f"wt{t}")
                nc.scalar.activation(wt[:], scs[t][:], Exp, scale=0.125)
                wts[t] = wt
                scs.pop(t - LAG, None)
                piece()
                if extra:
                    extra.pop(0)()
                if t >= LAG and (not defer or t - LAG >= DEFER):
                    av(t - LAG)
                piece()
            if defer:
                # defer the avs of the piece-congested window (tiles 0..15)
                # AND the final-tile flush into the next pass, which has PE
                # slack and no pieces (the flush would stall PE on this
                # pass's last exps right when the next pass's scores could
                # run).  Keep tiles 28..31 last so each region's stop flag
                # still closes its accumulation group.
                return [
                    (lambda tt: lambda: av(tt))(t)
                    for t in list(range(DEFER)) + list(range(NK - LAG, NK))
                ]
            if tail is None:
                for t in range(NK - LAG, NK):
                    av(t)
            else:
                # finish accB's accumulation first so its out_block (on ACT)
                # runs in parallel with accA's remaining avs + DVE out_block
                cb_b, cb_a = tail
                for t in range(NK - LAG, NK):
                    av_js(t, 4, 8)
                cb_b()
                for t in range(NK - LAG, NK):
                    av_js(t, 0, 4)
                cb_a()

        def out_block(acc, blk, on_act=False):
            # blk in 0..3: output rows 512*blk .. 512*blk+512.  The final
            # (tail) blocks split the multiplies between ACT and DVE.
            res = stage.tile([128, 4, H], F32, tag="res")
            for j in range(4):
                rec = stage.tile([128, 1], F32, tag="rec")
                nc.vector.reciprocal(rec[:], acc[:, j, 64:65])
                if on_act:
                    nc.scalar.activation(
                        res[:, j, :], acc[:, j, 0:64], Copy, scale=rec[:]
                    )
                else:
                    nc.vector.tensor_scalar_mul(res[:, j, :], acc[:, j, 0:64], rec[:])
            nc.sync.dma_start(
                OUT[512 * blk : 512 * (blk + 1), :].rearrange("(j p) c -> p j c", p=128),
                res[:],
            )

        accA = accp.tile([128, 4, 128], F32, tag="acc", name="acc0A")
        accB = accp.tile([128, 4, 128], F32, tag="acc", name="acc0B")
        av_defer = attn_pass(0, accA, accB, interleave=True, head=True, defer=True)
        assert not pieces, f"{len(pieces)} deferred pieces never emitted"
        if dbg:
            qtd = dbgp.tile([64, SQ], F32, tag="qtd")
            nc.vector.tensor_copy(qtd[:], qT[:])
            nc.sync.dma_start(QTD[:], qtd[:])
            ktd = dbgp.tile([64, S], F32, tag="ktd")
            nc.vector.tensor_copy(ktd[:], kT[:])
            nc.sync.dma_start(KTD[:], ktd[:])
            vod = dbgp.tile([128, NK * (H + 1)], F32, tag="vod")
            nc.vector.tensor_copy(vod[:], vones[:].rearrange("p a b -> p (a b)"))
            nc.sync.dma_start(VOD[:], vod[:])
            accd = dbgp.tile([128, 4 * 128], F32, tag="accd")
            nc.vector.tensor_copy(accd[:], accA[:].rearrange("p a b -> p (a b)"))
            nc.sync.dma_start(ACCD[:], accd[:])
        av_defer.append(lambda: out_block(accA, 0))
        av_defer.append(lambda: out_block(accB, 1))

        # pass-1 accumulators live in the (now idle) pjp banks so pass 0's
        # stay valid while its deferred avs drain inside pass 1
        acc1A = pjp.tile([128, 4, 128], F32, tag="pj", name="acc1A")
        acc1B = pjp.tile([128, 4, 128], F32, tag="pj", name="acc1B")
        res8 = stage.tile([128, 8, H], F32, tag="res8", name="res8")

        def tail_half(acc, r0, dve_only):
            # reciprocals first, then multiply pairs with the pair's output
            # DMA issued immediately (overlapping the DMA-issue pipeline with
            # the remaining normalize work).  The first (critical-path) half
            # keeps everything on DVE — one queue, no cross-engine semaphore
            # hops; the second half splits DVE/ACT for parallelism.
            rec4 = stage.tile([128, 4], F32, tag="rec4", name=f"rec4_{r0}")
            for j in range(4):
                nc.vector.reciprocal(rec4[:, j : j + 1], acc[:, j, 64:65])
            for jp in (0, 2):
                nc.vector.tensor_scalar_mul(
                    res8[:, r0 + jp, :], acc[:, jp, 0:64], rec4[:, jp : jp + 1]
                )
                if dve_only:
                    nc.vector.tensor_scalar_mul(
                        res8[:, r0 + jp + 1, :],
                        acc[:, jp + 1, 0:64],
                        rec4[:, jp + 1 : jp + 2],
                    )
                else:
                    nc.scalar.activation(
                        res8[:, r0 + jp + 1, :],
                        acc[:, jp + 1, 0:64],
                        Copy,
                        scale=rec4[:, jp + 1 : jp + 2],
                    )
                row = 1024 + 128 * (r0 + jp)
                nc.sync.dma_start(
                    OUT[row : row + 256, :].rearrange("(j p) c -> p j c", p=128),
                    res8[:, r0 + jp : r0 + jp + 2, :],
                )

        def tail_b():
            tail_half(acc1B, 4, dve_only=True)

        def tail_a():
            tail_half(acc1A, 0, dve_only=False)

        attn_pass(
            1, acc1A, acc1B, interleave=False, extra=av_defer, tail=(tail_b, tail_a)
        )
        assert not av_defer, f"{len(av_defer)} deferred avs never emitted"

    nc.compile()
    return nc


def kernel(hidden_states, Wq, bq, Wk, bk, Wv, bv):
    global _NC, LAST_RESULT
    import ml_dtypes
    from concourse.bass_utils import run_bass_kernel_spmd

    BF = ml_dtypes.bfloat16
    hidden_states = np.asarray(hidden_states, dtype=np.float32)
    Wq = np.asarray(Wq, dtype=np.float32)
    Wk = np.asarray(Wk, dtype=np.float32)
    Wv = np.asarray(Wv, dtype=np.float32)
    bq = np.asarray(bq, dtype=np.float32)
    bk = np.asarray(bk, dtype=np.float32)
    bv = np.asarray(bv, dtype=np.float32)
    B = hidden_states.shape[0]
    assert hidden_states.shape == (4, S, E), hidden_states.shape

    if _NC is None:
        _NC = _build()

    wkv = np.concatenate([Wk, Wv], axis=1)  # [E, 128]
    wkv_t = np.ascontiguousarray(
        wkv.reshape(NT, 128, 2 * H).transpose(1, 0, 2).astype(BF)
    )
    wq_t = np.ascontiguousarray(Wq.reshape(NT, 128, H).transpose(1, 0, 2).astype(BF))
    bkv = np.concatenate([bk, bv]).reshape(2 * H, 1).copy()
    bq1 = bq.reshape(H, 1).copy()

    in_maps = []
    for core in range(N_CORES):
        b, half = divmod(core, 2)
        q0 = half * SQ
        hid_rot = np.roll(hidden_states[b], -q0, axis=0)  # [S, E]
        hidt = np.ascontiguousarray(
            hid_rot.reshape(S, NT, 128).transpose(2, 0, 1).astype(BF)
        )  # [128, S, NT]
        in_maps.append({"hidt": hidt, "wkv": wkv_t, "wq": wq_t, "bkv": bkv, "bq": bq1})

    LAST_RESULT = run_bass_kernel_spmd(_NC, in_maps, core_ids=list(range(N_CORES)))
    out = np.empty((B, S, H), np.float32)
    for core in range(N_CORES):
        b, half = divmod(core, 2)
        q0 = half * SQ
        out[b, q0 : q0 + SQ] = LAST_RESULT.results[core]["out"]
    return out
